# revision 1
# baseline (speedup 1.0000x reference)
"""Trainium2 Bass kernel for nn_GCNTime (GCN + per-t causal transformer over nodes).

Sharding: T=16 time steps across 8 cores (2 per core). The graph (dense
normalized adjacency) is replicated; every stage is independent across t,
so there are no collectives.

Per-core layout: activations are feature-major [h=128 partitions, token]
(token = t_local*2048 + node). All linear layers run with the weight as the
stationary matmul operand. GCN aggregation uses node-major x blocks as the
stationary operand against the transposed adjacency as the moving operand,
which yields feature-major output directly. Attention scores are computed
transposed (keys on partitions) so exp(scores) feeds the A@V matmul without
transposition; softmax denominators and layernorm statistics are
partition-axis reductions done with a ones-matrix matmul.
"""

import math
from contextlib import ExitStack

import numpy as np
import ml_dtypes

import concourse.bacc as bacc
import concourse.tile as tile
from concourse import mybir
from concourse.bass_utils import run_bass_kernel_spmd

P = 128
N = 2048          # nodes
T = 16            # total time steps
TL = 2            # time steps per core
NB = N // P       # node blocks (16)
H = 128
DFF = 2048
NF = DFF // P     # ffn chunks (16)
L = 2
TOKS = TL * N     # tokens per core (4096)
NDC = N // 512    # 512-wide node chunks (4)
TC = TOKS // 512  # 512-wide token chunks (8)
EPS = 1e-5
SCALE = 1.0 / math.sqrt(H)
SQH = math.sqrt(H)

f32 = mybir.dt.float32
f32r = mybir.dt.float32r
bf16 = mybir.dt.bfloat16
bfnp = ml_dtypes.bfloat16

AF = mybir.ActivationFunctionType
ALU = mybir.AluOpType


def _emit(tc, io):
    nc = tc.nc
    with ExitStack() as ctx:
        consts = ctx.enter_context(tc.tile_pool(name="consts", bufs=1))
        spool = ctx.enter_context(tc.tile_pool(name="spool", bufs=5))
        xpool = ctx.enter_context(tc.tile_pool(name="xpool", bufs=1))
        resid = ctx.enter_context(tc.tile_pool(name="resid", bufs=2))
        actbf = ctx.enter_context(tc.tile_pool(name="actbf", bufs=2))
        qkvp = ctx.enter_context(tc.tile_pool(name="qkvp", bufs=2))
        apool = ctx.enter_context(tc.tile_pool(name="apool", bufs=2))
        ffp = ctx.enter_context(tc.tile_pool(name="ffp", bufs=4))
        small = ctx.enter_context(tc.tile_pool(name="small", bufs=4))
        ost = ctx.enter_context(tc.tile_pool(name="ost", bufs=4))
        psum = ctx.enter_context(tc.tile_pool(name="psum", bufs=8, space="PSUM"))

        dma = nc.sync.dma_start
        wdma = nc.gpsimd.dma_start

        # ---- load constants / weights ----
        ident_f = consts.tile([P, P], f32, tag="ident_f")
        wdma(out=ident_f, in_=io["ident_f"])
        ones_b = consts.tile([P, P], bf16, tag="ones_b")
        wdma(out=ones_b, in_=io["ones_b"])
        masks = consts.tile([P, 4, 512], bf16, tag="masks")
        wdma(out=masks, in_=io["masks"].rearrange("m p x -> p m x"))
        pe_t = consts.tile([P, TL], f32, tag="pe_t")
        wdma(out=pe_t, in_=io["pe_t"])
        eps_t = consts.tile([P, 1], f32, tag="eps_t")
        nc.vector.memset(eps_t, EPS)
        ones_f = consts.tile([P, P], f32r, tag="ones_f")
        wdma(out=ones_f, in_=io["ones_f"])

        # layer-0 node-major input: emit DMA early so aggregation starts first
        xnm = xpool.tile([P, TL * N], bf16, tag="xnm", name="xnm0_v2")
        dma(out=xnm.rearrange("p (t c f) -> p t c f", t=TL, c=NB),
            in_=io["x_nm"].rearrange("t (c p) f -> p t c f", p=P))

        wl = []
        for l in range(L):
            d = {}
            d["convW"] = consts.tile([P, H], bf16, tag=f"convW{l}", name=f"convW{l}")
            wdma(out=d["convW"], in_=io["convW"][l])
            d["conv_b"] = consts.tile([P, 1], f32, tag=f"conv_b{l}", name=f"conv_b{l}")
            wdma(out=d["conv_b"], in_=io["conv_b"][l])
            d["wqkvT"] = consts.tile([P, 3 * H], bf16, tag=f"wqkvT{l}", name=f"wqkvT{l}")
            wdma(out=d["wqkvT"], in_=io["wqkvT"][l])
            d["bqkv"] = consts.tile([P, 3], f32, tag=f"bqkv{l}", name=f"bqkv{l}")
            wdma(out=d["bqkv"], in_=io["bqkv"][l])
            d["bqkv_vr"] = consts.tile([1, H], bf16, tag=f"bqkv_vr{l}", name=f"bqkv_vr{l}")
            wdma(out=d["bqkv_vr"], in_=io["bqkv_vr"][l])
            d["woT"] = consts.tile([P, H], bf16, tag=f"woT{l}", name=f"woT{l}")
            wdma(out=d["woT"], in_=io["woT"][l])
            d["bo"] = consts.tile([P, 1], f32, tag=f"bo{l}", name=f"bo{l}")
            wdma(out=d["bo"], in_=io["bo"][l])
            d["w1T"] = consts.tile([P, DFF], bf16, tag=f"w1T{l}", name=f"w1T{l}")
            wdma(out=d["w1T"], in_=io["w1T"][l])
            d["b1"] = consts.tile([P, NF], f32, tag=f"b1{l}", name=f"b1{l}")
            wdma(out=d["b1"], in_=io["b1"][l])
            d["w2T"] = consts.tile([P, NF, H], bf16, tag=f"w2T{l}", name=f"w2T{l}")
            wdma(out=d["w2T"], in_=io["w2T"][l].rearrange("c p h -> p c h"))
            d["b2"] = consts.tile([P, 1], f32, tag=f"b2{l}", name=f"b2{l}")
            wdma(out=d["b2"], in_=io["b2"][l])
            for nm in ("ln1g", "ln1b", "ln2g", "ln2b"):
                d[nm] = consts.tile([P, 1], f32, tag=f"{nm}{l}", name=f"{nm}{l}")
                wdma(out=d[nm], in_=io[nm][l])
            wl.append(d)

        for l in range(L):
            w = wl[l]

            # ================= GCN aggregation: agg[f, d] = sum_s x[s,f] * S[d,s]
            ps_agg = []
            for t in range(TL):
                for dc in range(NDC):
                    pa = psum.tile([P, 512], f32, tag="ps", name=f"agg{l}_{t}_{dc}")
                    ps_agg.append(pa)
            for c in range(NB):
                s_tile = spool.tile([P, N], bf16, tag="s_tile", name=f"s{l}_{c}")
                (dma if c % 2 == 0 else nc.scalar.dma_start)(
                    out=s_tile, in_=io["s_t"][c])
                for t in range(TL):
                    for dc in range(NDC):
                        nc.tensor.matmul(
                            ps_agg[t * NDC + dc],
                            xnm[:, (t * NB + c) * P:(t * NB + c + 1) * P],
                            s_tile[:, dc * 512:(dc + 1) * 512],
                            start=(c == 0), stop=(c == NB - 1),
                        )
            aggb = actbf.tile([P, TOKS], bf16, tag="aggb", bufs=1, name=f"aggb_v2_{l}")
            for t in range(TL):
                for dc in range(NDC):
                    osl = aggb[:, (t * NDC + dc) * 512:(t * NDC + dc + 1) * 512]
                    if dc % 2 == 0:
                        nc.scalar.copy(osl, ps_agg[t * NDC + dc])
                    else:
                        nc.vector.tensor_copy(osl, ps_agg[t * NDC + dc])

            # ================= GCN linear + relu + *sqrt(H) + posenc -> h1 (residual)
            h1 = resid.tile([P, TOKS], f32, tag="resid", name=f"h1_{l}")
            h1b = actbf.tile([P, TOKS], bf16, tag="h1b", name=f"h1b{l}")
            for tch in range(TC):
                sl = slice(tch * 512, (tch + 1) * 512)
                pc = psum.tile([P, 512], f32, tag="ps", name=f"conv{l}_{tch}")
                nc.tensor.matmul(pc, w["convW"], aggb[:, sl], start=True, stop=True)
                nc.scalar.activation(
                    out=h1[:, sl], in_=pc, func=AF.Relu, bias=w["conv_b"])
                t = tch // NDC
                nc.vector.tensor_scalar(
                    out=h1[:, sl], in0=h1[:, sl],
                    scalar1=SQH, scalar2=pe_t[:, t:t + 1], op0=ALU.mult, op1=ALU.add)
                nc.vector.tensor_copy(h1b[:, sl], h1[:, sl])

            # ================= attention: qkv for both t, then (ic, t)-interleaved
            z1 = resid.tile([P, TOKS], f32r, tag="resid", name=f"z1_{l}")
            qs, ks, vns, attns = [], [], [], []
            for t in range(TL):
                q_t = qkvp.tile([P, N], bf16, tag="q_t", name=f"q{l}_{t}")
                k_t = qkvp.tile([P, N], bf16, tag="k_t", name=f"k{l}_{t}")
                for part, dest in ((0, q_t), (1, k_t)):
                    for ncc in range(NDC):
                        sl = slice(ncc * 512, (ncc + 1) * 512)
                        pq = psum.tile([P, 512], f32, tag="ps", name=f"qkv{l}_{t}_{part}_{ncc}")
                        nc.tensor.matmul(
                            pq, w["wqkvT"][:, part * H:(part + 1) * H],
                            h1b[:, t * N + ncc * 512: t * N + (ncc + 1) * 512],
                            start=True, stop=True)
                        nc.scalar.activation(
                            out=dest[:, sl], in_=pq, func=AF.Identity,
                            bias=w["bqkv"][:, part:part + 1])
                # v computed node-major directly: lhsT = h1b node-block
                # (stationary), rhs = Wv (moving); bias added as a rank-1
                # K=1 matmul accumulation (ones row x bias row)
                vn = qkvp.tile([P, NB, P], bf16, tag="vn", bufs=2, name=f"vn{l}_{t}")
                for j in range(NB):
                    pt = psum.tile([P, 512], f32, tag="ps", name=f"vtr{l}_{t}_{j}")
                    hsl = h1b[:, t * N + j * P: t * N + (j + 1) * P]
                    nc.tensor.matmul(pt[:, :P], hsl,
                                     w["wqkvT"][:, 2 * H:3 * H],
                                     start=True, stop=False)
                    nc.tensor.matmul(pt[:, :P], ones_b[0:1, :],
                                     w["bqkv_vr"], start=False, stop=True)
                    nc.vector.tensor_copy(vn[:, j, :], pt[:, :P])
                attnb = qkvp.tile([P, N], bf16, tag="attnb", bufs=2, name=f"attnb{l}_{t}")
                qs.append(q_t); ks.append(k_t); vns.append(vn); attns.append(attnb)

            for ic in range(NDC):
                for t in range(TL):
                    q_t, k_t, vn, attnb = qs[t], ks[t], vns[t], attns[t]
                    jmax = 4 * ic + 4
                    qsl = slice(ic * 512, (ic + 1) * 512)
                    A = apool.tile([P, NB, 512], bf16, tag="A", name=f"A{l}_{t}_{ic}")
                    for j in range(jmax):
                        pa = psum.tile([P, 512], f32, tag="ps", name=f"sc{l}_{t}_{ic}_{j}")
                        nc.tensor.matmul(
                            pa, k_t[:, j * P:(j + 1) * P], q_t[:, qsl],
                            start=True, stop=True)
                        nc.scalar.activation(
                            out=A[:, j, :], in_=pa, func=AF.Exp, scale=SCALE)
                        r = j - 4 * ic
                        if r >= 0:
                            nc.vector.tensor_mul(A[:, j, :], A[:, j, :], masks[:, r, :])
                    pd = psum.tile([P, 512], f32, tag="ps", name=f"dn{l}_{t}_{ic}")
                    for j in range(jmax):
                        nc.tensor.matmul(pd, ones_b, A[:, j, :],
                                         start=(j == 0), stop=(j == jmax - 1))
                    rec = small.tile([P, 512], f32, tag="rec", bufs=2, name=f"rec{l}_{t}_{ic}")
                    nc.vector.reciprocal(rec, pd)
                    pv = psum.tile([P, 512], f32, tag="ps", name=f"av{l}_{t}_{ic}")
                    for j in range(jmax):
                        nc.tensor.matmul(pv, vn[:, j, :], A[:, j, :],
                                         start=(j == 0), stop=(j == jmax - 1))
                    nc.vector.tensor_tensor(
                        out=attnb[:, qsl], in0=pv, in1=rec, op=ALU.mult)
                    # Wo + bias + residual for this 512-token chunk
                    po = psum.tile([P, 512], f32, tag="ps", name=f"wo{l}_{t}_{ic}")
                    nc.tensor.matmul(po, w["woT"], attnb[:, qsl],
                                     start=True, stop=True)
                    sl = slice(t * N + ic * 512, t * N + (ic + 1) * 512)
                    nc.vector.scalar_tensor_tensor(
                        out=z1[:, sl], in0=po, scalar=w["bo"], in1=h1[:, sl],
                        op0=ALU.add, op1=ALU.add)

            # ================= LN1
            yl1 = resid.tile([P, TOKS], f32, tag="resid", name=f"yl1_{l}")
            yl1b = actbf.tile([P, TOKS], bf16, tag="h1b", name=f"yl1b{l}")
            self_ln(tc, psum, small, ones_f, eps_t, z1, yl1, yl1b,
                    w["ln1g"], w["ln1b"], f"ln1_{l}")

            # ================= FFN
            z2 = resid.tile([P, TOKS], f32r, tag="resid", name=f"z2_{l}")
            for tch in range(TC):
                sl = slice(tch * 512, (tch + 1) * 512)
                p2 = psum.tile([P, 512], f32, tag="ps", name=f"ff2{l}_{tch}")
                for c in range(NF):
                    p1 = psum.tile([P, 512], f32, tag="ps", name=f"ff1{l}_{tch}_{c}")
                    nc.tensor.matmul(p1, w["w1T"][:, c * P:(c + 1) * P],
                                     yl1b[:, sl], start=True, stop=True)
                    f1 = ffp.tile([P, 512], bf16, tag="f1", name=f"f1_{l}_{tch}_{c}")
                    if c % 2 == 0:
                        nc.scalar.activation(
                            out=f1, in_=p1, func=AF.Relu, bias=w["b1"][:, c:c + 1])
                    else:
                        nc.vector.tensor_scalar(
                            out=f1, in0=p1, scalar1=w["b1"][:, c:c + 1], scalar2=0.0,
                            op0=ALU.add, op1=ALU.max)
                    nc.tensor.matmul(p2, w["w2T"][:, c, :], f1,
                                     start=(c == 0), stop=(c == NF - 1))
                nc.vector.scalar_tensor_tensor(
                    out=z2[:, sl], in0=p2, scalar=w["b2"], in1=yl1[:, sl],
                    op0=ALU.add, op1=ALU.add)

            # ================= LN2
            yo = resid.tile([P, TOKS], f32, tag="resid", name=f"yo_{l}")
            self_ln(tc, psum, small, ones_f, eps_t, z2, yo, None,
                    w["ln2g"], w["ln2b"], f"ln2_{l}")

            if l < L - 1:
                # next layer's node-major input via fp32 PE transpose of yo
                # (4 transposed blocks per psum tile, one evac copy each)
                xnm = xpool.tile([P, TL * N], bf16, tag="xnm", name=f"xnm{l + 1}")
                for t in range(TL):
                    for g in range(NB // 4):
                        pt = psum.tile([P, 512], f32, tag="ps", name=f"xtr{l}_{t}_{g}")
                        for k in range(4):
                            c = g * 4 + k
                            nc.tensor.transpose(
                                pt[:, k * P:(k + 1) * P],
                                yo[:, (t * NB + c) * P:(t * NB + c + 1) * P],
                                ident_f)
                        nc.vector.tensor_copy(
                            xnm[:, (t * NB + g * 4) * P:(t * NB + g * 4 + 4) * P], pt)
            else:
                # final output: transpose fp32 to node-major, DMA out
                # (4 transposed blocks per psum tile / staging copy / DMA)
                for t in range(TL):
                    for g in range(NB // 4):
                        pt = psum.tile([P, 512], f32, tag="ps", name=f"otr{t}_{g}")
                        for k in range(4):
                            c = g * 4 + k
                            nc.tensor.transpose(
                                pt[:, k * P:(k + 1) * P],
                                yo[:, (t * NB + c) * P:(t * NB + c + 1) * P],
                                ident_f)
                        stg = ost.tile([P, 4, P], f32, tag="stg", name=f"stg{t}_{g}")
                        nc.vector.tensor_copy(stg, pt.rearrange("p (c f) -> p c f", c=4))
                        dma(out=io["y"][t].rearrange("(g c p) f -> g p c f", g=4, c=4)[g],
                            in_=stg)


def self_ln(tc, psum, small, ones_f, eps_t, z, yout, youtb, g_ap, b_ap, nm):
    """LayerNorm over the partition (feature) axis of z [P, TOKS] (f32).

    Stats via fp32r ones-matmul partition reduction (full-rate at N=512).
    varq carries musq -> var -> sd -> rr in place; zc carries the centered
    value -> normalized value in place. Output written f32 (residual) and
    bf16 (matmul operand).
    """
    nc = tc.nc
    for tch in range(TC):
        sl = slice(tch * 512, (tch + 1) * 512)
        sq = small.tile([P, 512], f32r, tag="lnbf", bufs=3, name=f"sq_{nm}_{tch}")
        nc.scalar.activation(out=sq, in_=z[:, sl], func=AF.Square)
        p1 = psum.tile([P, 512], f32, tag="ps", name=f"lns_{nm}_{tch}")
        nc.tensor.matmul(p1, ones_f, z[:, sl], start=True, stop=True)
        p2 = psum.tile([P, 512], f32, tag="ps", name=f"lnq_{nm}_{tch}")
        nc.tensor.matmul(p2, ones_f, sq, start=True, stop=True)
        # zc = z - sum(z)/P   (reads the sum psum directly)
        zc = small.tile([P, 512], f32, tag="lntmp", bufs=6, name=f"zc_{nm}_{tch}")
        nc.vector.scalar_tensor_tensor(
            out=zc, in0=p1, scalar=-1.0 / P, in1=z[:, sl],
            op0=ALU.mult, op1=ALU.add)
        # varq: mean(z)^2 -> var -> sqrt(var+eps) -> rstd, all in place
        varq = small.tile([P, 512], f32, tag="lntmp", bufs=6, name=f"varq_{nm}_{tch}")
        nc.scalar.activation(out=varq, in_=p1, func=AF.Square, scale=1.0 / P)
        nc.vector.scalar_tensor_tensor(
            out=varq, in0=p2, scalar=1.0 / P, in1=varq,
            op0=ALU.mult, op1=ALU.subtract)
        nc.scalar.activation(out=varq, in_=varq, func=AF.Sqrt, bias=eps_t)
        nc.vector.reciprocal(varq, varq)
        # zc = (z - mu) * g * rstd, in place
        nc.vector.scalar_tensor_tensor(
            out=zc, in0=zc, scalar=g_ap, in1=varq, op0=ALU.mult, op1=ALU.mult)
        nc.vector.tensor_scalar(
            out=yout[:, sl], in0=zc, scalar1=b_ap, scalar2=None, op0=ALU.add)
        if youtb is not None:
            nc.gpsimd.tensor_scalar(
                out=youtb[:, sl], in0=zc, scalar1=b_ap, scalar2=None, op0=ALU.add)


_CACHE = {}


def _build():
    if "nc" in _CACHE:
        return _CACHE["nc"], _CACHE["io_names"]
    nc = bacc.Bacc("TRN2", target_bir_lowering=False, debug=False, num_devices=8)
    io = {}

    def inp(name, shape, dt):
        io[name] = nc.dram_tensor(name, shape, dt, kind="ExternalInput").ap()

    inp("x_nm", [TL, N, H], bf16)
    inp("s_t", [NB, P, N], bf16)
    inp("pe_t", [P, TL], f32)
    inp("convW", [L, P, H], bf16)
    inp("conv_b", [L, P, 1], f32)
    inp("wqkvT", [L, P, 3 * H], bf16)
    inp("bqkv", [L, P, 3], f32)
    inp("bqkv_vr", [L, 1, H], bf16)
    inp("woT", [L, P, H], bf16)
    inp("bo", [L, P, 1], f32)
    inp("w1T", [L, P, DFF], bf16)
    inp("b1", [L, P, NF], f32)
    inp("w2T", [L, NF, P, H], bf16)
    inp("b2", [L, P, 1], f32)
    inp("ln1g", [L, P, 1], f32)
    inp("ln1b", [L, P, 1], f32)
    inp("ln2g", [L, P, 1], f32)
    inp("ln2b", [L, P, 1], f32)
    inp("masks", [4, P, 512], bf16)
    inp("ident_f", [P, P], f32)
    inp("ones_b", [P, P], bf16)
    inp("ones_f", [P, P], f32r)
    inp("vtag", [1, 27], f32)
    io["y"] = nc.dram_tensor("y", [TL, N, H], f32, kind="ExternalOutput").ap()

    with tile.TileContext(nc) as t:
        _emit(t, io)
    nc.compile()
    _CACHE["nc"] = nc
    _CACHE["io_names"] = list(io)
    return nc, list(io)


def _host_prep(inputs):
    """Build the shared (replicated) device arrays from the full inputs."""
    x = np.asarray(inputs["x"], np.float32)
    edge = np.asarray(inputs["edge_index"])

    src = np.concatenate([edge[0], np.arange(N, dtype=edge.dtype)])
    dst = np.concatenate([edge[1], np.arange(N, dtype=edge.dtype)])
    deg = np.zeros(N, np.float32)
    np.add.at(deg, dst, 1.0)
    dinv = 1.0 / np.sqrt(deg)
    normv = (dinv[src] * dinv[dst]).astype(np.float32)
    S = np.zeros((N, N), np.float32)
    np.add.at(S, (dst, src), normv)
    s_t = np.ascontiguousarray(S.T.reshape(NB, P, N)).astype(bfnp)

    pos = np.arange(T, dtype=np.float32)[:, None]
    ii = np.arange(0, H, 2, dtype=np.float32)
    pes = np.sin(pos / (10000.0 ** (2.0 * ii / H))).astype(np.float32)
    pec = np.cos(pos / (10000.0 ** (2.0 * (ii + 1.0) / H))).astype(np.float32)
    pe = np.stack([pes, pec], axis=-1).reshape(T, H).astype(np.float32)

    conv_W = np.asarray(inputs["conv_W"], np.float32)
    Wqkv = np.asarray(inputs["Wqkv"], np.float32)
    Wo = np.asarray(inputs["Wo"], np.float32)
    W1 = np.asarray(inputs["W1"], np.float32)
    W2 = np.asarray(inputs["W2"], np.float32)

    shared = {
        "s_t": s_t,
        "convW": conv_W.astype(bfnp),
        "conv_b": np.asarray(inputs["conv_b"], np.float32).reshape(L, P, 1),
        "wqkvT": np.ascontiguousarray(Wqkv.transpose(0, 2, 1)).astype(bfnp),
        "bqkv": np.ascontiguousarray(
            np.asarray(inputs["bqkv"], np.float32).reshape(L, 3, P).transpose(0, 2, 1)),
        "bqkv_vr": np.asarray(inputs["bqkv"], np.float32).reshape(
            L, 3, P)[:, 2:3, :].astype(bfnp),
        "woT": np.ascontiguousarray(Wo.transpose(0, 2, 1)).astype(bfnp),
        "bo": np.asarray(inputs["bo"], np.float32).reshape(L, P, 1),
        "w1T": np.ascontiguousarray(W1.transpose(0, 2, 1)).astype(bfnp),
        "b1": np.ascontiguousarray(
            np.asarray(inputs["b1"], np.float32).reshape(L, NF, P).transpose(0, 2, 1)),
        "w2T": np.ascontiguousarray(
            W2.transpose(0, 2, 1).reshape(L, NF, P, H)).astype(bfnp),
        "b2": np.asarray(inputs["b2"], np.float32).reshape(L, P, 1),
        "ln1g": np.asarray(inputs["ln1_g"], np.float32).reshape(L, P, 1),
        "ln1b": np.asarray(inputs["ln1_b"], np.float32).reshape(L, P, 1),
        "ln2g": np.asarray(inputs["ln2_g"], np.float32).reshape(L, P, 1),
        "ln2b": np.asarray(inputs["ln2_b"], np.float32).reshape(L, P, 1),
        "masks": (np.arange(512)[None, None, :] >=
                  (np.arange(4)[:, None, None] * P + np.arange(P)[None, :, None])
                  ).astype(bfnp),
        "ident_f": np.eye(P, dtype=np.float32),
        "ones_b": np.ones((P, P), np.float32).astype(bfnp),
        "ones_f": np.ones((P, P), np.float32),
        "vtag": np.zeros((1, 27), np.float32),
    }
    return shared, x, pe


def kernel(**inputs):
    nc, _ = _build()
    shared, x, pe = _host_prep(inputs)

    in_maps = []
    for core in range(8):
        t0 = core * TL
        m = dict(shared)
        m["x_nm"] = np.ascontiguousarray(
            x[:, t0:t0 + TL, :].transpose(1, 0, 2)).astype(bfnp)
        m["pe_t"] = np.ascontiguousarray(pe[t0:t0 + TL].T)
        in_maps.append(m)

    res = run_bass_kernel_spmd(nc, in_maps, list(range(8)))

    out = np.zeros((N, T, H), np.float32)
    for core in range(8):
        t0 = core * TL
        out[:, t0:t0 + TL, :] = res.results[core]["y"].transpose(1, 0, 2)
    return out



# revision 64
# speedup vs baseline: 5.7097x; 5.7097x over previous
"""Trainium2 Bass kernel for nn_GCNTime (GCN + per-t causal transformer over nodes).

Sharding: T=16 time steps across 8 cores (2 per core). The graph (dense
normalized adjacency) is replicated; every stage is independent across t,
so there are no collectives.

v2 layout notes (per core):
- Activations are feature-major [128 partitions, token] (token = t*2048+node).
- PSUM is managed as three tag pools: "big"/"st" [128,1024] (two banks) and
  "ps" [128,512]; paired 1024-wide tiles halve evac/exp instruction counts.
- Attention scores are computed transposed (keys on partitions); the causal
  block-triangle is trimmed at 128-column granularity and only the single
  diagonal 128x128 sub-block per key-block needs a mask multiply.
- Softmax denominator and LN statistics are partition reductions via
  ones-matmuls; LN uses Act Rsqrt (no DVE reciprocal).
- Layer0 -> layer1 node-major input is produced by DMA XBAR transposes
  (bf16), not PE transposes. The final output stays feature-major and is
  transposed on the host.
- Weights arrive as two packed DMAs (bf16 + f32).
"""

import math
from contextlib import ExitStack

import numpy as np
import ml_dtypes

import concourse.bacc as bacc
import concourse.tile as tile
from concourse import mybir
from concourse.bass_utils import run_bass_kernel_spmd

P = 128
N = 2048          # nodes
T = 16            # total time steps
TL = 2            # time steps per core
NB = N // P       # node blocks (16)
H = 128
DFF = 2048
NF = DFF // P     # ffn chunks (16)
L = 2
TOKS = TL * N     # tokens per core (4096)
NDC = N // 512    # 512-wide node chunks (4)
TC = TOKS // 512  # 512-wide token chunks (8)
EPS = 1e-5
SCALE = 1.0 / math.sqrt(H)
SQH = math.sqrt(H)

f32 = mybir.dt.float32
f32r = mybir.dt.float32r
bf16 = mybir.dt.bfloat16
bfnp = ml_dtypes.bfloat16

AF = mybir.ActivationFunctionType
ALU = mybir.AluOpType

# ---- packed weight column offsets (bf16 pack) ----
# per layer: convW H | wqkvT 3H | woT H | w1T DFF | w2T NF*H
_LW = H + 3 * H + H + DFF + NF * H          # 4736
BCOLS = L * _LW + 4 * P                      # + ones_b + negI + slt + negrow
NEG_BIG = float(2 ** 20)
# f32 pack per layer: convb_s 1 | bq 1 | bk 1 | bo 1 | b1 NF | b2 1 |
#                     ln1g 1 | ln1b 1 | ln2g 1 | ln2b 1
_LF = 8 + NF
FCOLS = L * _LF + 1                          # + eps


def _boffs(l):
    o = l * _LW
    return {
        "convW": o, "wqkvT": o + H, "woT": o + 4 * H,
        "w1T": o + 5 * H, "w2T": o + 5 * H + DFF,
    }


def _foffs(l):
    o = l * _LF
    return {
        "convb_s": o, "bq": o + 1, "bk": o + 2, "bo": o + 3,
        "b1": o + 4, "b2": o + 4 + NF, "ln1g": o + 5 + NF,
        "ln1b": o + 6 + NF, "ln2g": o + 7 + NF, "ln2b": o + 8 + NF,
    }


def _emit(tc, io, lean):
    nc = tc.nc
    with ExitStack() as ctx:
        consts = ctx.enter_context(tc.tile_pool(name="consts", bufs=1))
        spool = ctx.enter_context(tc.tile_pool(name="spool", bufs=14))
        xpool = ctx.enter_context(tc.tile_pool(name="xpool", bufs=1))
        resid = ctx.enter_context(tc.tile_pool(name="resid", bufs=2))
        hbp = ctx.enter_context(tc.tile_pool(name="hbp", bufs=2))
        qkvp = ctx.enter_context(tc.tile_pool(name="qkvp", bufs=2))
        apool = ctx.enter_context(tc.tile_pool(name="apool", bufs=3))
        ffp = ctx.enter_context(tc.tile_pool(name="ffp", bufs=3))
        small = ctx.enter_context(tc.tile_pool(name="small", bufs=2))
        psum = ctx.enter_context(tc.tile_pool(name="psum", bufs=2, space="PSUM"))

        sdma = nc.sync.dma_start
        adma = nc.scalar.dma_start
        wdma = nc.gpsimd.dma_start

        # ---- initial DMAs: critical data first (xnm t-halves + s_t c=0,1),
        # then packed weights on the software-DGE queue ----
        xnm = xpool.tile([P, TL, NB, P], bf16, tag="xnm", name="xnm0")
        sdma(out=xnm[:, 0, 0:4], in_=io["x_nm"][:, 0, 0:4])
        s_pre = []
        s0 = spool.tile([P, N], bf16, tag="s_tile", name="s0_0")
        adma(out=s0[:, 0:1024], in_=io["s_t"][0][:, 0:1024])
        sdma(out=xnm[:, 0, 4:NB], in_=io["x_nm"][:, 0, 4:NB])
        adma(out=s0[:, 1024:2048], in_=io["s_t"][0][:, 1024:2048])
        s_pre.append(s0)
        s1 = spool.tile([P, N], bf16, tag="s_tile", name="s0_1")
        adma(out=s1, in_=io["s_t"][1])
        sdma(out=xnm[:, 1], in_=io["x_nm"][:, 1])
        s_pre.append(s1)

        # wb pieces are DMA'd interleaved with the layer-0 s_t stream (below)
        # so the big weight transfers don't block the startup-critical loads
        wb = consts.tile([P, BCOLS], bf16, tag="wb")
        wb_pieces = []
        for l in range(L):
            o = l * _LW
            wb_pieces.append((o, 5 * H))              # convW|wqkvT|woT
            wb_pieces.append((o + 5 * H, DFF))        # w1T
            wb_pieces.append((o + 5 * H + DFF, NF * H))  # w2T
        wb_pieces.append((L * _LW, 4 * P))            # ones|negI|slt|negrow
        wf = consts.tile([P, FCOLS], f32, tag="wf")
        wdma(out=wf, in_=io["wf"])
        pe_t = consts.tile([P, TL], f32, tag="pe_t")
        wdma(out=pe_t, in_=io["pe_t"])
        ones_f = consts.tile([P, P], f32r, tag="ones_f")
        wdma(out=ones_f, in_=io["ones_f"])
        bvr = consts.tile([1, L * H], bf16, tag="bvr")
        wdma(out=bvr, in_=io["bvr"])

        ones_b = wb[:, L * _LW:L * _LW + P]
        negI = wb[:, L * _LW + P:L * _LW + 2 * P]
        slt = wb[:, L * _LW + 2 * P:L * _LW + 3 * P]
        negrow = wb[:, L * _LW + 3 * P:L * _LW + 4 * P]
        eps_t = wf[:, L * _LF:L * _LF + 1]

        for l in range(L):
            bo_ = _boffs(l)
            fo = _foffs(l)
            convW = wb[:, bo_["convW"]:bo_["convW"] + H]
            wqkvT = wb[:, bo_["wqkvT"]:bo_["wqkvT"] + 3 * H]
            woT = wb[:, bo_["woT"]:bo_["woT"] + H]
            w1T = wb[:, bo_["w1T"]:bo_["w1T"] + DFF]
            w2T = wb[:, bo_["w2T"]:bo_["w2T"] + NF * H]
            convb_s = wf[:, fo["convb_s"]:fo["convb_s"] + 1]
            bq = wf[:, fo["bq"]:fo["bq"] + 1]
            bk = wf[:, fo["bk"]:fo["bk"] + 1]
            bo_ap = wf[:, fo["bo"]:fo["bo"] + 1]
            b1 = wf[:, fo["b1"]:fo["b1"] + NF]
            b2 = wf[:, fo["b2"]:fo["b2"] + 1]
            ln1g = wf[:, fo["ln1g"]:fo["ln1g"] + 1]
            ln1b = wf[:, fo["ln1b"]:fo["ln1b"] + 1]
            ln2g = wf[:, fo["ln2g"]:fo["ln2g"] + 1]
            ln2b = wf[:, fo["ln2b"]:fo["ln2b"] + 1]

            # ======== GCN aggregation: agg[f, tok] = sum_s x[s,f] * S^T[s, dst]
            # 5 accumulator psums covering [t, dc]: big(t0,dc01) big(t0,dc23)
            # st(t1,dc01) ps(t1,dc2) ps(t1,dc3)
            agA = psum.tile([P, 1024], f32, tag="big", name=f"agA{l}")
            agB = psum.tile([P, 1024], f32, tag="big", name=f"agB{l}")
            agC = psum.tile([P, 1024], f32, tag="st", bufs=1, name=f"agC{l}")
            agD = psum.tile([P, 512], f32, tag="pA", bufs=1, name=f"agD{l}")
            agE = psum.tile([P, 512], f32, tag="pB", bufs=1, name=f"agE{l}")
            dsts = [
                (0, agA[:, 0:512]), (0, agA[:, 512:1024]),
                (0, agB[:, 0:512]), (0, agB[:, 512:1024]),
                (1, agC[:, 0:512]), (1, agC[:, 512:1024]),
                (1, agD), (1, agE),
            ]
            for c in range(NB):
                if l == 0:
                    if c < 2:
                        s_tile = s_pre[c]
                    else:
                        s_tile = spool.tile([P, N], bf16, tag="s_tile",
                                            name=f"s{l}_{c}")
                        (sdma if c % 2 == 0 else adma)(
                            out=s_tile, in_=io["s_t"][c])
                    if 1 <= c <= len(wb_pieces):
                        off, w = wb_pieces[c - 1]
                        (sdma if c % 2 == 1 else adma)(
                            out=wb[:, off:off + w], in_=io["wb"][:, off:off + w])
                else:
                    s_tile = s_next[c]
                for i, (t, dst) in enumerate(dsts):
                    dc = i % 4
                    nc.tensor.matmul(
                        dst, xnm[:, t, c], s_tile[:, dc * 512:(dc + 1) * 512],
                        start=(c == 0), stop=(c == NB - 1))
            if l < L - 1:
                # prefetch next layer's adjacency tiles during this layer's
                # compute (DMA engines are otherwise idle mid-layer)
                s_next = []
                for c in range(NB):
                    st_ = spool.tile([P, N], bf16, tag="s_tile",
                                     name=f"s{l + 1}_{c}")
                    sdma(out=st_, in_=io["s_t"][c])
                    s_next.append(st_)
            aggb = hbp.tile([P, TOKS], bf16, tag="hb", name=f"aggb{l}")
            nc.scalar.copy(aggb[:, 0:1024], agA)
            nc.vector.tensor_copy(aggb[:, 1024:2048], agB)
            nc.vector.tensor_copy(aggb[:, 2048:3072], agC)
            nc.scalar.copy(aggb[:, 3072:3584], agD)
            nc.vector.tensor_copy(aggb[:, 3584:4096], agE)

            # ======== GCN linear -> relu*sqrt(H) -> +pe -> h1b (bf16 residual)
            h1b = hbp.tile([P, TOKS], bf16, tag="hb", name=f"h1b{l}")
            for pr in range(TC // 2):
                t = (2 * pr) // NDC
                pc = psum.tile([P, 1024], f32, tag="big", name=f"conv{l}_{pr}")
                sl = slice(pr * 1024, (pr + 1) * 1024)
                nc.tensor.matmul(pc[:, 0:512], convW,
                                 aggb[:, pr * 1024:pr * 1024 + 512],
                                 start=True, stop=True)
                nc.tensor.matmul(pc[:, 512:1024], convW,
                                 aggb[:, pr * 1024 + 512:(pr + 1) * 1024],
                                 start=True, stop=True)
                nc.scalar.activation(out=h1b[:, sl], in_=pc, func=AF.Relu,
                                     scale=SQH, bias=convb_s)
                nc.gpsimd.tensor_scalar(
                    out=h1b[:, sl], in0=h1b[:, sl], scalar1=pe_t[:, t:t + 1],
                    scalar2=None, op0=ALU.add)

            # ======== qkv (q,k feature-major; v node-major)
            qs, ks, vns, attns = [], [], [], []
            for t in range(TL):
                q_t = qkvp.tile([P, N], bf16, tag="q_t", name=f"q{l}_{t}")
                k_t = qkvp.tile([P, N], bf16, tag="k_t", name=f"k{l}_{t}")
                for part, dest, bias in ((0, q_t, bq), (1, k_t, bk)):
                    for hp in range(2):
                        pq = psum.tile([P, 1024], f32, tag="big",
                                       name=f"qk{l}_{t}_{part}_{hp}")
                        for hh in range(2):
                            ncc = hp * 2 + hh
                            nc.tensor.matmul(
                                pq[:, hh * 512:(hh + 1) * 512],
                                wqkvT[:, part * H:(part + 1) * H],
                                h1b[:, t * N + ncc * 512:t * N + (ncc + 1) * 512],
                                start=True, stop=True)
                        osl = dest[:, hp * 1024:(hp + 1) * 1024]
                        if part == 0:
                            nc.vector.tensor_scalar(
                                out=osl, in0=pq, scalar1=bias,
                                scalar2=None, op0=ALU.add)
                        else:
                            nc.scalar.activation(
                                out=osl, in_=pq, func=AF.Identity, bias=bias)
                vn = qkvp.tile([P, NB, P], bf16, tag="vn", name=f"vn{l}_{t}")
                for hp in range(2):
                    pv = psum.tile([P, 1024], f32, tag="big",
                                   name=f"v{l}_{t}_{hp}")
                    for j in range(8):
                        jj = hp * 8 + j
                        reg = pv[:, j * P:(j + 1) * P]
                        nc.tensor.matmul(
                            reg,
                            h1b[:, t * N + jj * P:t * N + (jj + 1) * P],
                            wqkvT[:, 2 * H:3 * H],
                            start=True, stop=lean)
                        if not lean:
                            # bias via rank-1 accumulation (ones row x bias row)
                            nc.tensor.matmul(
                                reg, ones_b[0:1, :],
                                bvr[:, l * H:(l + 1) * H],
                                start=False, stop=True)
                    nc.vector.tensor_copy(
                        vn[:, hp * 8:(hp + 1) * 8, :], pv)
                attnb = qkvp.tile([P, N], bf16, tag="attnb", name=f"at{l}_{t}")
                qs.append(q_t); ks.append(k_t); vns.append(vn); attns.append(attnb)

            # ======== attention, (ic, t)-interleaved, causal-trimmed
            z1 = resid.tile([P, TOKS], f32r, tag="resid", name=f"z1_{l}")
            yl1b = hbp.tile([P, TOKS], bf16, tag="hb", name=f"yl1b{l}")
            ci = 0
            for ic in range(NDC):
                for t in range(TL):
                    q_t, k_t, vn, attnb = qs[t], ks[t], vns[t], attns[t]
                    jmax = 4 * ic + 4
                    npair = jmax // 2
                    q0 = ic * 512

                    def rs(j):
                        return 128 * max(0, j - 4 * ic)

                    # pd/pv double-buffer across chunks: even chunks use the
                    # pA/pB half-banks, odd chunks the st pair
                    if ci % 2 == 0:
                        pd = psum.tile([P, 512], f32, tag="pA", bufs=1,
                                       name=f"pd{l}_{t}_{ic}")
                        pv = psum.tile([P, 512], f32, tag="pB", bufs=1,
                                       name=f"pv{l}_{t}_{ic}")
                    else:
                        pdpv = psum.tile([P, 1024], f32, tag="st", bufs=1,
                                         name=f"pdpv{l}_{t}_{ic}")
                        pd, pv = pdpv[:, 0:512], pdpv[:, 512:1024]
                    pairs = []

                    def emit_dnav(p):
                        A_p, j0 = pairs[p]
                        for h in range(2):
                            j = j0 + h
                            r0 = rs(j)
                            nc.tensor.matmul(
                                pd[:, r0:512], ones_b,
                                A_p[:, h * 512 + r0:(h + 1) * 512],
                                start=(j == 0), stop=(j == jmax - 1))
                        for h in range(2):
                            j = j0 + h
                            r0 = rs(j)
                            nc.tensor.matmul(
                                pv[:, r0:512], vn[:, j, :],
                                A_p[:, h * 512 + r0:(h + 1) * 512],
                                start=(j == 0), stop=(j == jmax - 1))

                    for p in range(npair):
                        j0 = 2 * p
                        pa = psum.tile([P, 1024], f32, tag="big",
                                       name=f"sc{l}_{t}_{ic}_{p}")
                        diag = j0 >= 4 * ic
                        for h in range(2):
                            j = j0 + h
                            r0 = rs(j)
                            nc.tensor.matmul(
                                pa[:, h * 512 + r0:(h + 1) * 512],
                                k_t[:, j * P:(j + 1) * P],
                                q_t[:, q0 + r0:q0 + 512],
                                start=True, stop=not diag,
                                skip_group_check=diag)
                            if diag:
                                # causal mask: add -BIG to the upper triangle
                                # of the diagonal 128x128 sub-block (exp -> 0)
                                nc.tensor.matmul(
                                    pa[:, h * 512 + r0:h * 512 + r0 + 128],
                                    negI, slt, start=False, stop=True,
                                    skip_group_check=True)
                        A_p = apool.tile([P, 1024], bf16, tag="A",
                                         name=f"A{l}_{t}_{ic}_{p}")
                        r0 = rs(j0)
                        nc.scalar.activation(out=A_p[:, r0:1024],
                                             in_=pa[:, r0:1024],
                                             func=AF.Exp, scale=SCALE)
                        pairs.append((A_p, j0))
                        if p >= 2:
                            emit_dnav(p - 2)
                    if npair >= 2:
                        emit_dnav(npair - 2)
                    emit_dnav(npair - 1)

                    rec = small.tile([P, 512], f32, tag="rec", bufs=2,
                                     name=f"rec{l}_{t}_{ic}")
                    nc.vector.reciprocal(rec, pd)
                    nc.vector.tensor_tensor(
                        out=attnb[:, q0:q0 + 512], in0=pv,
                        in1=rec, op=ALU.mult)
                    po = psum.tile([P, 512], f32,
                                   tag=("pA" if ci % 2 == 0 else "pB"), bufs=1,
                                   name=f"wo{l}_{t}_{ic}")
                    nc.tensor.matmul(po, woT, attnb[:, q0:q0 + 512],
                                     start=True, stop=True)
                    sl = slice(t * N + q0, t * N + q0 + 512)
                    nc.vector.scalar_tensor_tensor(
                        out=z1[:, sl], in0=po, scalar=bo_ap, in1=h1b[:, sl],
                        op0=ALU.add, op1=ALU.add)
                    ci += 1
                    if ci == 4:
                        # hoist the first two LN1 chunks (their z1 tokens are
                        # complete) so the LN chain overlaps late attention
                        for e in range(2):
                            self_ln(tc, psum, small, ones_f, eps_t, z1, yl1b,
                                    None, ln1g, ln1b, lean, f"ln1_{l}", e)

            # ======== FFN (+ remaining LN1) -> z2 ; then LN2
            z2 = resid.tile([P, TOKS], f32r, tag="resid", name=f"z2_{l}")


            def emit_ffn(tch):
                sl = slice(tch * 512, (tch + 1) * 512)
                p2 = psum.tile([P, 512], f32, tag="pA", bufs=1,
                               name=f"ff2{l}_{tch}")
                f1s = []

                def emit_w2(cp):
                    f1 = f1s[cp]
                    for h in range(2):
                        c = 2 * cp + h
                        nc.tensor.matmul(
                            p2, w2T[:, c * P:(c + 1) * P],
                            f1[:, h * 512:(h + 1) * 512],
                            start=(c == 0), stop=(c == NF - 1))

                for cp in range(NF // 2):
                    p1 = psum.tile([P, 1024], f32, tag="big",
                                   name=f"ff1{l}_{tch}_{cp}")
                    for h in range(2):
                        c = 2 * cp + h
                        nc.tensor.matmul(
                            p1[:, h * 512:(h + 1) * 512],
                            w1T[:, c * P:(c + 1) * P], yl1b[:, sl],
                            start=True, stop=True)
                    f1 = ffp.tile([P, 1024], bf16, tag="f1",
                                  name=f"f1_{l}_{tch}_{cp}")
                    eng = 0 if cp % 8 in (0, 1, 2, 4, 6) else 1
                    if lean:
                        if eng == 0:
                            nc.scalar.activation(out=f1, in_=p1, func=AF.Relu)
                        else:
                            nc.vector.tensor_scalar(
                                out=f1, in0=p1, scalar1=0.0, scalar2=None,
                                op0=ALU.max)
                    else:
                        for h in range(2):
                            c = 2 * cp + h
                            hs = slice(h * 512, (h + 1) * 512)
                            if eng == 0:
                                nc.scalar.activation(
                                    out=f1[:, hs], in_=p1[:, hs], func=AF.Relu,
                                    bias=b1[:, c:c + 1])
                            else:
                                nc.vector.tensor_scalar(
                                    out=f1[:, hs], in0=p1[:, hs],
                                    scalar1=b1[:, c:c + 1], scalar2=0.0,
                                    op0=ALU.add, op1=ALU.max)
                    f1s.append(f1)
                    if cp >= 1:
                        emit_w2(cp - 1)
                emit_w2(NF // 2 - 1)
                nc.vector.scalar_tensor_tensor(
                    out=z2[:, sl], in0=p2, scalar=b2, in1=yl1b[:, sl],
                    op0=ALU.add, op1=ALU.add)

            # LN1 / FFN / LN2 software-pipelined per 512-token chunk; LN2
            # feeds the next layer's node-major input (DMA transpose) or the
            # final feature-major output DMA.
            if l < L - 1:
                youtb = hbp.tile([P, TOKS], bf16, tag="hb", name=f"ynb{l}")
                xnm = xpool.tile([P, TL, NB, P], bf16, tag="xnm", name="xnm1")
                yo = None
            else:
                youtb = None

            def emit_ln2(tch):
                if youtb is not None:
                    self_ln(tc, psum, small, ones_f, eps_t, z2, youtb, None,
                            ln2g, ln2b, lean, f"ln2_{l}", tch)
                    t, ic = tch // NDC, tch % NDC
                    nc.sync.dma_start_transpose(
                        xnm[:, t, 4 * ic:4 * ic + 4, :],
                        youtb[:, tch * 512:(tch + 1) * 512])
                else:
                    yo_c = small.tile([P, 512], f32, tag="yoc", bufs=2,
                                      name=f"yoc{tch}")
                    dq = sdma if tch % 2 == 0 else adma
                    if tch == TC - 1:
                        # split the last chunk for a shorter drain chain
                        for hh in range(2):
                            self_ln(tc, psum, small, ones_f, eps_t, z2, None,
                                    yo_c, ln2g, ln2b, lean, f"ln2_{l}_h{hh}",
                                    tch, col0=tch * 512 + hh * 256, W=256)
                            sl = slice(tch * 512 + hh * 256,
                                       tch * 512 + (hh + 1) * 256)
                            (sdma if hh == 0 else adma)(
                                out=io["y"][:, sl],
                                in_=yo_c[:, hh * 256:(hh + 1) * 256])
                    else:
                        self_ln(tc, psum, small, ones_f, eps_t, z2, None, yo_c,
                                ln2g, ln2b, lean, f"ln2_{l}", tch)
                        sl = slice(tch * 512, (tch + 1) * 512)
                        dq(out=io["y"][:, sl], in_=yo_c)

            for step in range(TC):
                if step + 2 < TC:
                    self_ln(tc, psum, small, ones_f, eps_t, z1, yl1b, None,
                            ln1g, ln1b, lean, f"ln1_{l}", step + 2)
                emit_ffn(step)
                emit_ln2(step)


def self_ln(tc, psum, small, ones_f, eps_t, z, youtb, youtf, g_ap, b_ap,
            lean, nm, tch, col0=None, W=512):
    """LayerNorm chunk over the partition (feature) axis of z [P, TOKS].

    Stats via fp32r ones-matmul partition reduction into a paired psum
    (p1 = sum | p2 = sumsq); output written bf16 (youtb) or f32 (youtf).
    """
    nc = tc.nc
    if col0 is None:
        col0 = tch * 512
    sl = slice(col0, col0 + W)
    sq = small.tile([P, 512], f32r, tag="sq", bufs=2, name=f"sq_{nm}_{tch}")
    nc.gpsimd.tensor_tensor(out=sq[:, 0:W], in0=z[:, sl], in1=z[:, sl],
                            op=ALU.mult)
    pst = psum.tile([P, 1024], f32, tag="st", bufs=1, name=f"lns_{nm}_{tch}")
    nc.tensor.matmul(pst[:, 0:W], ones_f, z[:, sl], start=True, stop=True)
    nc.tensor.matmul(pst[:, 512:512 + W], ones_f, sq[:, 0:W],
                     start=True, stop=True)
    p1, p2 = pst[:, 0:W], pst[:, 512:512 + W]
    # musq = (p1/P)^2 on Pool; varq = p2/P - musq on DVE
    musq = small.tile([P, 512], f32, tag="musq", bufs=2, name=f"mu_{nm}_{tch}")
    nc.scalar.activation(out=musq[:, 0:W], in_=p1, func=AF.Square,
                         scale=1.0 / P)
    varq = small.tile([P, 512], f32, tag="varq", bufs=2, name=f"va_{nm}_{tch}")
    nc.vector.scalar_tensor_tensor(
        out=varq[:, 0:W], in0=p2, scalar=1.0 / P, in1=musq[:, 0:W],
        op0=ALU.mult, op1=ALU.subtract)
    nc.scalar.activation(out=varq[:, 0:W], in_=varq[:, 0:W], func=AF.Sqrt,
                         bias=eps_t)
    nc.vector.reciprocal(varq[:, 0:W], varq[:, 0:W])
    zc = small.tile([P, 512], f32, tag="zc", bufs=2, name=f"zc_{nm}_{tch}")
    nc.vector.scalar_tensor_tensor(
        out=zc[:, 0:W], in0=p1, scalar=-1.0 / P, in1=z[:, sl],
        op0=ALU.mult, op1=ALU.add)
    out = youtb if youtb is not None else youtf
    osl = out[:, sl] if out.shape[-1] == TOKS else out[:, col0 - tch * 512:
                                                      col0 - tch * 512 + W]
    if lean:
        # g == 1, b == 0: out = zc * rstd as a plain Pool tensor-tensor
        nc.gpsimd.tensor_tensor(out=osl, in0=zc[:, 0:W], in1=varq[:, 0:W],
                                op=ALU.mult)
    else:
        nc.vector.scalar_tensor_tensor(
            out=zc[:, 0:W], in0=zc[:, 0:W], scalar=g_ap, in1=varq[:, 0:W],
            op0=ALU.mult, op1=ALU.mult)
        nc.vector.tensor_scalar(out=osl, in0=zc[:, 0:W], scalar1=b_ap,
                                scalar2=None, op0=ALU.add)


_CACHE = {}


def _build(lean=True):
    key = ("nc", lean)
    if key in _CACHE:
        return _CACHE[key]
    nc = bacc.Bacc("TRN2", target_bir_lowering=False, debug=False, num_devices=8)
    io = {}

    def inp(name, shape, dt):
        io[name] = nc.dram_tensor(name, shape, dt, kind="ExternalInput").ap()

    inp("x_nm", [P, TL, NB, H], bf16)
    inp("s_t", [NB, P, N], bf16)
    inp("pe_t", [P, TL], f32)
    inp("wb", [P, BCOLS], bf16)
    inp("wf", [P, FCOLS], f32)
    inp("ones_f", [P, P], f32r)
    inp("bvr", [1, L * H], bf16)
    io["y"] = nc.dram_tensor("y", [P, TOKS], f32, kind="ExternalOutput").ap()

    with tile.TileContext(nc) as t:
        _emit(t, io, lean)
    nc.compile()
    _CACHE[key] = (nc, list(io))
    return nc, list(io)


def _host_prep(inputs):
    """Build the shared (replicated) device arrays from the full inputs."""
    x = np.asarray(inputs["x"], np.float32)
    edge = np.asarray(inputs["edge_index"])

    src = np.concatenate([edge[0], np.arange(N, dtype=edge.dtype)])
    dst = np.concatenate([edge[1], np.arange(N, dtype=edge.dtype)])
    deg = np.zeros(N, np.float32)
    np.add.at(deg, dst, 1.0)
    dinv = 1.0 / np.sqrt(deg)
    normv = (dinv[src] * dinv[dst]).astype(np.float32)
    S = np.zeros((N, N), np.float32)
    np.add.at(S, (dst, src), normv)
    s_t = np.ascontiguousarray(S.T.reshape(NB, P, N)).astype(bfnp)

    pos = np.arange(T, dtype=np.float32)[:, None]
    ii = np.arange(0, H, 2, dtype=np.float32)
    pes = np.sin(pos / (10000.0 ** (2.0 * ii / H))).astype(np.float32)
    pec = np.cos(pos / (10000.0 ** (2.0 * (ii + 1.0) / H))).astype(np.float32)
    pe = np.stack([pes, pec], axis=-1).reshape(T, H).astype(np.float32)

    conv_W = np.asarray(inputs["conv_W"], np.float32)
    Wqkv = np.asarray(inputs["Wqkv"], np.float32)
    Wo = np.asarray(inputs["Wo"], np.float32)
    W1 = np.asarray(inputs["W1"], np.float32)
    W2 = np.asarray(inputs["W2"], np.float32)
    bqkv = np.asarray(inputs["bqkv"], np.float32).reshape(L, 3, P)
    conv_b = np.asarray(inputs["conv_b"], np.float32)
    b1v = np.asarray(inputs["b1"], np.float32).reshape(L, NF, P)

    wbp = np.zeros((P, BCOLS), np.float32)
    wfp = np.zeros((P, FCOLS), np.float32)
    for l in range(L):
        b = _boffs(l)
        f = _foffs(l)
        wbp[:, b["convW"]:b["convW"] + H] = conv_W[l]
        wbp[:, b["wqkvT"]:b["wqkvT"] + 3 * H] = Wqkv[l].T
        wbp[:, b["woT"]:b["woT"] + H] = Wo[l].T
        wbp[:, b["w1T"]:b["w1T"] + DFF] = W1[l].T
        wbp[:, b["w2T"]:b["w2T"] + NF * H] = (
            W2[l].T.reshape(NF, P, H).transpose(1, 0, 2).reshape(P, NF * H))
        wfp[:, f["convb_s"]] = conv_b[l] * SQH
        wfp[:, f["bq"]] = bqkv[l, 0]
        wfp[:, f["bk"]] = bqkv[l, 1]
        wfp[:, f["bo"]] = np.asarray(inputs["bo"], np.float32)[l]
        wfp[:, f["b1"]:f["b1"] + NF] = b1v[l].T
        wfp[:, f["b2"]] = np.asarray(inputs["b2"], np.float32)[l]
        wfp[:, f["ln1g"]] = np.asarray(inputs["ln1_g"], np.float32)[l]
        wfp[:, f["ln1b"]] = np.asarray(inputs["ln1_b"], np.float32)[l]
        wfp[:, f["ln2g"]] = np.asarray(inputs["ln2_g"], np.float32)[l]
        wfp[:, f["ln2b"]] = np.asarray(inputs["ln2_b"], np.float32)[l]
    wfp[:, L * _LF] = EPS
    o = L * _LW
    wbp[:, o:o + P] = 1.0                                   # ones_b
    wbp[:, o + P:o + 2 * P] = -NEG_BIG * np.eye(P)          # negI
    wbp[:, o + 2 * P:o + 3 * P] = (                         # slt[k, v] = v < k
        np.arange(P)[None, :] < np.arange(P)[:, None]).astype(np.float32)
    wbp[:, o + 3 * P:o + 4 * P] = -NEG_BIG                  # negrow (rank-1 kill)
    # v-bias rows (one [1,H] row per layer for the rank-1 bias matmul)
    bvr = bqkv[:, 2, :].reshape(1, L * H)

    lean = all(
        not np.asarray(inputs[k], np.float32).any()
        for k in ("conv_b", "bqkv", "bo", "b1", "b2", "ln1_b", "ln2_b")
    ) and all(
        np.all(np.asarray(inputs[k], np.float32) == 1.0)
        for k in ("ln1_g", "ln2_g"))

    shared = {
        "s_t": s_t,
        "wb": wbp.astype(bfnp),
        "wf": wfp,
        "ones_f": np.ones((P, P), np.float32),
    }
    return shared, x, pe, bvr.astype(bfnp), lean


def make_in_maps(inputs):
    shared, x, pe, bvr, lean = _host_prep(inputs)
    shared["bvr"] = bvr
    in_maps = []
    for core in range(8):
        t0 = core * TL
        m = dict(shared)
        m["x_nm"] = np.ascontiguousarray(
            x[:, t0:t0 + TL, :].reshape(NB, P, TL, H).transpose(1, 2, 0, 3)
        ).astype(bfnp)
        m["pe_t"] = np.ascontiguousarray(pe[t0:t0 + TL].T)
        in_maps.append(m)
    return in_maps, lean


def kernel(**inputs):
    in_maps, lean = make_in_maps(inputs)
    nc, _ = _build(lean)
    res = run_bass_kernel_spmd(nc, in_maps, list(range(8)))

    out = np.zeros((N, T, H), np.float32)
    for core in range(8):
        t0 = core * TL
        yf = res.results[core]["y"].reshape(P, TL, N)
        for t in range(TL):
            out[:, t0 + t, :] = yf[:, t, :].T
    return out
